# revision 4
# baseline (speedup 1.0000x reference)
"""Trainium2 Bass kernel for GNN message passing:

    messages = e @ W_e.T + (h @ W_hu.T)[src] + (h @ W_hw.T)[tgt]

Strategy (8 NeuronCores, edge-parallel):
  - Edges are sharded 100k per core; h and the three weight matrices are
    replicated.
  - Phase 1 (per core): project the full node table once,
    hu = h @ W_hu.T and hw = h @ W_hw.T, into internal DRAM tables.
  - Phase 2 (per core): for each 128-edge tile, compute ee = e @ W_e.T on
    the tensor engine, then two indirect (gather) DMAs with an inline
    CCE add accumulate hu[src] and hw[tgt] straight into the message
    tile, which is then stored to DRAM.

Host-side prep only reshapes/transposes inputs (so the device sees
matmul-ready layouts) and slices edges per core; all FLOPs and gathers
run on device in fp32.
"""
import os
from contextlib import ExitStack

import numpy as np

import concourse.bass as bass
import concourse.tile as tile
from concourse import bacc, mybir
from concourse.bass_utils import run_bass_kernel_spmd

N_NODES = 50000
N_EDGES = 800000
IN_DIM = 128
OUT_DIM = 128
EDGE_DIM = 64
NCORES = 8

P = 128
NODES_PAD = 50176          # 98 * 512 = 392 * 128
EPC = N_EDGES // NCORES    # 100000 edges per core
BLK = 2048                 # edges per e-block load
NBLK = 49                  # blocks per core
EPC_PAD = NBLK * BLK       # 100352
TILES = EPC_PAD // P       # 784 tiles of 128 edges

F32 = mybir.dt.float32
I32 = mybir.dt.int32

_CACHE = {}
LAST = {}


def _build():
    nc = bacc.Bacc(
        "TRN2",
        target_bir_lowering=False,
        debug=False,
        enable_asserts=True,
        num_devices=NCORES,
    )

    hT = nc.dram_tensor("hT", [P, NODES_PAD], F32, kind="ExternalInput").ap()
    Wcat = nc.dram_tensor("Wcat", [P, 2 * OUT_DIM], F32, kind="ExternalInput").ap()
    WeT = nc.dram_tensor("WeT", [2 * EDGE_DIM, OUT_DIM], F32, kind="ExternalInput").ap()
    eP = nc.dram_tensor("eP", [NBLK, P, BLK // 2], F32, kind="ExternalInput").ap()
    sidx = nc.dram_tensor("sidx", [P, TILES], I32, kind="ExternalInput").ap()
    tidx = nc.dram_tensor("tidx", [P, TILES], I32, kind="ExternalInput").ap()
    msgs = nc.dram_tensor("msgs", [EPC_PAD, OUT_DIM], F32, kind="ExternalOutput").ap()

    hu_tab = nc.dram_tensor("hu_tab", [NODES_PAD, OUT_DIM], F32).ap()
    hw_tab = nc.dram_tensor("hw_tab", [NODES_PAD, OUT_DIM], F32).ap()

    with tile.TileContext(nc) as tc:
        with ExitStack() as ctx:
            wpool = ctx.enter_context(tc.tile_pool(name="w", bufs=1))
            idxpool = ctx.enter_context(tc.tile_pool(name="idx", bufs=1))
            hpool = ctx.enter_context(tc.tile_pool(name="h", bufs=3))
            p1psum = ctx.enter_context(tc.tile_pool(name="p1psum", bufs=4, space="PSUM"))
            opool = ctx.enter_context(tc.tile_pool(name="o", bufs=4))
            epool = ctx.enter_context(tc.tile_pool(name="e", bufs=3))
            p2psum = ctx.enter_context(tc.tile_pool(name="p2psum", bufs=4, space="PSUM"))
            mpool = ctx.enter_context(tc.tile_pool(name="m", bufs=16))

            wcat_t = wpool.tile([P, 2 * OUT_DIM], F32)
            nc.sync.dma_start(out=wcat_t[:], in_=Wcat[:])
            wet_t = wpool.tile([2 * EDGE_DIM, OUT_DIM], F32)
            nc.sync.dma_start(out=wet_t[:], in_=WeT[:])
            sidx_t = idxpool.tile([P, TILES], I32)
            nc.sync.dma_start(out=sidx_t[:], in_=sidx[:])
            tidx_t = idxpool.tile([P, TILES], I32)
            nc.sync.dma_start(out=tidx_t[:], in_=tidx[:])

            # ---- Phase 1: hu/hw node tables -------------------------------
            for i in range(NODES_PAD // 512):
                hb = hpool.tile([P, 512], F32)
                nc.sync.dma_start(out=hb[:], in_=hT[:, i * 512 : (i + 1) * 512])
                for s in range(4):
                    ps = p1psum.tile([P, 2 * OUT_DIM], F32)
                    nc.tensor.matmul(
                        out=ps[:],
                        lhsT=hb[:, s * P : (s + 1) * P],
                        rhs=wcat_t[:],
                        start=True,
                        stop=True,
                    )
                    ot = opool.tile([P, 2 * OUT_DIM], F32)
                    nc.vector.tensor_copy(out=ot[:], in_=ps[:])
                    n0 = (i * 4 + s) * P
                    nc.sync.dma_start(
                        out=hu_tab[n0 : n0 + P, :], in_=ot[:, :OUT_DIM]
                    )
                    nc.sync.dma_start(
                        out=hw_tab[n0 : n0 + P, :], in_=ot[:, OUT_DIM:]
                    )

            # ---- Phase 2: per-edge messages -------------------------------
            for b in range(NBLK):
                eb = epool.tile([P, BLK // 2], F32)
                nc.sync.dma_start(out=eb[:], in_=eP[b])
                for t in range(16):
                    tid = b * 16 + t
                    half = 0 if t < 8 else EDGE_DIM
                    col = (t % 8) * P
                    ps = p2psum.tile([P, OUT_DIM], F32)
                    nc.tensor.matmul(
                        out=ps[:],
                        lhsT=eb[half : half + EDGE_DIM, col : col + P],
                        rhs=wet_t[half : half + EDGE_DIM, :],
                        start=True,
                        stop=True,
                    )
                    mt = mpool.tile([P, OUT_DIM], F32)
                    nc.vector.tensor_copy(out=mt[:], in_=ps[:])
                    nc.gpsimd.indirect_dma_start(
                        out=mt[:],
                        out_offset=None,
                        in_=hu_tab[:],
                        in_offset=bass.IndirectOffsetOnAxis(
                            ap=sidx_t[:, tid : tid + 1], axis=0
                        ),
                        compute_op=mybir.AluOpType.add,
                    )
                    nc.gpsimd.indirect_dma_start(
                        out=mt[:],
                        out_offset=None,
                        in_=hw_tab[:],
                        in_offset=bass.IndirectOffsetOnAxis(
                            ap=tidx_t[:, tid : tid + 1], axis=0
                        ),
                        compute_op=mybir.AluOpType.add,
                    )
                    nc.sync.dma_start(
                        out=msgs[tid * P : (tid + 1) * P, :], in_=mt[:]
                    )

    nc.compile()
    return nc


def get_nc():
    if "nc" not in _CACHE:
        _CACHE["nc"] = _build()
    return _CACHE["nc"]


def _prep_in_maps(h, e, edge_index, W_e, W_hu, W_hw):
    h = np.ascontiguousarray(np.asarray(h, dtype=np.float32))
    e = np.ascontiguousarray(np.asarray(e, dtype=np.float32))
    src = np.asarray(edge_index[0]).astype(np.int32)
    tgt = np.asarray(edge_index[1]).astype(np.int32)
    W_e = np.asarray(W_e, dtype=np.float32)
    W_hu = np.asarray(W_hu, dtype=np.float32)
    W_hw = np.asarray(W_hw, dtype=np.float32)

    hT = np.zeros((P, NODES_PAD), dtype=np.float32)
    hT[:, :N_NODES] = h.T
    Wcat = np.ascontiguousarray(
        np.concatenate([W_hu.T, W_hw.T], axis=1)
    )  # [128, 256]
    # stacked twice so phase 2 has a copy at SBUF base partition 0 and 64
    WeT = np.ascontiguousarray(np.vstack([W_e.T, W_e.T]))  # [128, 128]

    in_maps = []
    for c in range(NCORES):
        sl = slice(c * EPC, (c + 1) * EPC)
        e_pad = np.zeros((EPC_PAD, EDGE_DIM), dtype=np.float32)
        e_pad[:EPC] = e[sl]
        ePc = np.ascontiguousarray(
            e_pad.reshape(NBLK, 2, BLK // 2, EDGE_DIM).transpose(0, 1, 3, 2)
        ).reshape(NBLK, P, BLK // 2)
        s_pad = np.zeros((EPC_PAD,), dtype=np.int32)
        s_pad[:EPC] = src[sl]
        t_pad = np.zeros((EPC_PAD,), dtype=np.int32)
        t_pad[:EPC] = tgt[sl]
        in_maps.append(
            {
                "hT": hT,
                "Wcat": Wcat,
                "WeT": WeT,
                "eP": ePc,
                "sidx": np.ascontiguousarray(s_pad.reshape(TILES, P).T),
                "tidx": np.ascontiguousarray(t_pad.reshape(TILES, P).T),
            }
        )
    return in_maps


def _install_ntff_hook():
    """Best-effort: register the axon NTFF profile hook when the image's
    antenv package lacks axon_hooks (needed only for trace=True runs)."""
    import sys
    import types

    try:
        from antenv.axon_hooks import get_axon_ntff_profile_hook  # noqa: F401

        return
    except ImportError:
        pass
    try:
        from trn_agent_boot.trn_boot import _ntff_profile_via_ctypes

        hook = _ntff_profile_via_ctypes("/opt/axon/libaxon_pjrt.so")
        mod = types.ModuleType("antenv.axon_hooks")
        mod._hook = hook
        mod.get_axon_ntff_profile_hook = lambda: mod._hook
        mod.set_axon_ntff_profile_hook = lambda h: setattr(mod, "_hook", h)
        sys.modules["antenv.axon_hooks"] = mod
        import antenv

        antenv.axon_hooks = mod
    except Exception:
        pass


def kernel(h, e, edge_index, W_e, W_hu, W_hw):
    nc = get_nc()
    in_maps = _prep_in_maps(h, e, edge_index, W_e, W_hu, W_hw)
    trace = bool(int(os.environ.get("KERNEL_TRACE", "0")))
    if trace:
        _install_ntff_hook()
    res = run_bass_kernel_spmd(nc, in_maps, list(range(NCORES)), trace=trace)
    LAST["exec_time_ns"] = res.exec_time_ns
    LAST["results"] = res
    out = np.empty((N_EDGES, OUT_DIM), dtype=np.float32)
    for c in range(NCORES):
        out[c * EPC : (c + 1) * EPC] = res.results[c]["msgs"][:EPC]
    return out
